# revision 1
# baseline (speedup 1.0000x reference)
"""CQT (constant-Q transform) kernel for Trainium2, 8 NeuronCores.

Math: out[b, c, t] = sum_l W[c, l] * x_pad[b, t*HOP + l]   (strided conv,
HOP=512, L=11339 taps, C=168 channels = 84 bins x re/im), then reshaped to
(B, 2, n_bins, T_out).

Strategy:
  - Data-parallel: shard B=32 across 8 cores (4 batches/core), weights
    replicated.
  - The conv is decomposed into 128-tap blocks: block i covers taps
    [128*i, 128*i+128).  For each block, out[c, t] += Wblk_i[:, c].T @
    X_i[:, t] is a matmul with K=128 on partitions.  The moving operand for
    block i=(4j+k) at output tile [t0, t0+NT) is a contiguous column slice
    of a host-pre-transposed view of x:  xt[r, k, u] = x_pad[512u+128k+r].
  - CQT kernels are ragged (bin k has ~11339*2^(-k/12) taps, centered), so
    the active channels of each block form a prefix; blocks run with
    M = (last nonzero channel + 1) only.  PSUM accumulates all blocks per
    output tile (bank0 = channels 0:128, bank1 = 128:C), fp32r matmuls
    (1 cycle/row at N>=256, FP22 multiply / FP32 accumulate).
"""

import numpy as np

HOP = 512
N_CORES = 8

_prog_cache: dict = {}


def _host_prep(x, kernels):
    x = np.ascontiguousarray(np.asarray(x, dtype=np.float32))
    kernels = np.ascontiguousarray(np.asarray(kernels, dtype=np.float32))
    B, T = x.shape
    nbins, two, Lmax = kernels.shape
    assert two == 2
    C = 2 * nbins
    pad = Lmax // 2
    T_out = (T + 2 * pad - Lmax) // HOP + 1

    # ---- weights: pad taps to 128 multiple, find ragged active prefixes ----
    nblk_full = -(-Lmax // 128)
    Wp = np.zeros((C, nblk_full * 128), dtype=np.float32)
    Wp[:, :Lmax] = kernels.reshape(C, Lmax)
    nz = (Wp.reshape(C, nblk_full, 128) != 0.0).any(axis=2)  # [C, nblk]
    Ms, keep = [], []
    for i in range(nblk_full):
        idx = np.where(nz[:, i])[0]
        if len(idx):
            keep.append(i)
            Ms.append(int(idx[-1]) + 1)
    keep = np.asarray(keep, dtype=np.int64)
    Ms = np.asarray(Ms, dtype=np.int64)
    # order blocks by descending active-channel count: the first matmul per
    # PSUM bank then covers the bank's maximal partition range (required for
    # the start=True zero-region semantics), and the weight DMA can be
    # chunked in exactly the order the matmuls consume it.
    order = np.argsort(-Ms, kind="stable")
    keep = keep[order]
    Ms = Ms[order]
    # Ragged SBUF weight layout: block pos stores only its M_pos active
    # channels: wt[r, offs[pos] + c] = Wp[c, 128*keep[pos] + r], c < M_pos.
    # (4.4x less weight traffic than storing all C channels per block.)
    wblk = Wp.reshape(C, nblk_full, 128)
    wt = np.ascontiguousarray(
        np.concatenate(
            [wblk[:m, i, :].T for i, m in zip(keep, Ms)], axis=1
        )
    )
    offs = np.concatenate([[0], np.cumsum(Ms)]).tolist()
    keep = keep.tolist()
    Ms = Ms.tolist()

    # ---- x: pad and pre-transpose to [128, 4, U] per batch ----
    j_max = int(max(keep)) // 4
    U = T_out + j_max
    xpad_len = 512 * U
    assert xpad_len >= pad + T, (xpad_len, pad + T)
    xp = np.zeros((B, xpad_len), dtype=np.float32)
    xp[:, pad:pad + T] = x
    # xt[b, r, k*U + u] = xp[b, 512u + 128k + r]
    xt = np.ascontiguousarray(
        xp.reshape(B, U, 4, 128).transpose(0, 3, 2, 1).reshape(B, 128, 4 * U)
    )
    return xt, wt, keep, Ms, offs, C, U, T_out, nbins


def _build_program(b_per, C, U, T_out, keep, Ms, offs):
    import concourse.mybir as mybir
    import concourse.tile as tile
    from concourse import bacc

    f32 = mybir.dt.float32
    f32r = mybir.dt.float32r
    nblk = len(keep)
    sum_m = offs[-1]
    mb_max = max(max(Ms) - 128, 0)
    nts = [512] * (T_out // 512) + ([T_out % 512] if T_out % 512 else [])
    # blocks already ordered by descending M in host prep
    a_ps = list(range(nblk))
    b_ps = [p for p in a_ps if Ms[p] > 128]
    j_max = max(keep) // 4
    # weight DMA chunks in matmul consumption order; first chunks small so
    # the first matmuls' dependencies land as early as possible
    w_budgets = [192, 256, 512] + [704] * nblk
    w_chunks = []
    p0 = 0
    while p0 < nblk:
        budget = w_budgets[len(w_chunks)]
        p1 = p0 + 1
        while p1 < nblk and offs[p1 + 1] - offs[p0] <= budget:
            p1 += 1
        w_chunks.append((p0, p1))
        p0 = p1
    # x DMA chunks: one per t-tile window (u-ranges, exclusive ends)
    x_stops = []
    t0 = 0
    for nt in nts:
        x_stops.append(min(t0 + nt + j_max + 1, U))
        t0 += nt
    x_stops[-1] = U
    x_chunks = []
    u0 = 0
    for u1 in x_stops:
        if u1 > u0:
            x_chunks.append((u0, u1))
            u0 = u1

    nc = bacc.Bacc(
        "TRN2",
        target_bir_lowering=False,
        debug=False,
        enable_asserts=True,
        num_devices=N_CORES,
    )
    xt_d = nc.dram_tensor("xt", [b_per, 128, 4 * U], f32r, kind="ExternalInput").ap()
    wt_d = nc.dram_tensor("wt", [128, sum_m], f32r, kind="ExternalInput").ap()
    out_d = nc.dram_tensor("out", [b_per, C, T_out], f32, kind="ExternalOutput").ap()

    with tile.TileContext(nc) as tc:
        with (
            tc.tile_pool(name="wpool", bufs=1) as wpool,
            tc.tile_pool(name="xpool", bufs=2) as xpool,
            tc.tile_pool(name="evpool", bufs=3) as evpool,
            tc.tile_pool(name="pspool", bufs=2, space="PSUM") as pspool,
        ):
            wsb = wpool.tile([128, sum_m], f32r)

            def dma_x_chunk(xb_tile, b, u0, u1, ks):
                # 3D AP: k-planes ks (a contiguous range), u in [u0, u1)
                src = xt_d[b].rearrange("r (k u) -> r k u", k=4)
                dst = xb_tile.rearrange("r (k u) -> r k u", k=4)
                nc.sync.dma_start(
                    out=dst[:, ks[0]:ks[-1] + 1, u0:u1],
                    in_=src[:, ks[0]:ks[-1] + 1, u0:u1],
                )

            # interleave first batch's x chunks with the weight chunks (both
            # in consumption order) so the first sweep's matmuls start after
            # ~1MB of DMA instead of ~10MB.  The very first x window is
            # split per k-plane in first-use order.
            xb0 = xpool.tile([128, 4 * U], f32r, tag="xb", name="xb0")
            k_first = []
            for p in a_ps:
                k = keep[p] % 4
                if k not in k_first:
                    k_first.append(k)
            x_emits = [(x_chunks[0], (k,)) for k in k_first]
            x_emits += [(ch, (0, 1, 2, 3)) for ch in x_chunks[1:]]
            emits = []
            for i in range(max(len(x_emits), len(w_chunks))):
                if i < len(x_emits):
                    emits.append(("x", x_emits[i]))
                if i < len(w_chunks):
                    emits.append(("w", w_chunks[i]))
            for kind, args in emits:
                if kind == "x":
                    (u0, u1), ks = args
                    dma_x_chunk(xb0, 0, u0, u1, ks)
                else:
                    a0, a1 = args
                    nc.sync.dma_start(
                        out=wsb[:, offs[a0]:offs[a1]],
                        in_=wt_d[:, offs[a0]:offs[a1]],
                    )

            for b in range(b_per):
                if b == 0:
                    xb = xb0
                else:
                    xb = xpool.tile([128, 4 * U], f32r, tag="xb", name=f"xb{b}")
                    nc.sync.dma_start(out=xb[:], in_=xt_d[b])
                t0 = 0
                for nt in nts:
                    pa = pspool.tile([128, 512], f32, tag="pa")
                    if mb_max:
                        pb = pspool.tile([128, 512], f32, tag="pb")
                    for pos, p in enumerate(a_ps):
                        m = Ms[p]
                        j, k = divmod(keep[p], 4)
                        rhs = xb[:, k * U + t0 + j: k * U + t0 + j + nt]
                        ma = min(m, 128)
                        nc.tensor.matmul(
                            pa[:ma, :nt],
                            lhsT=wsb[:, offs[p]: offs[p] + ma],
                            rhs=rhs,
                            start=(pos == 0),
                            stop=(pos == len(a_ps) - 1),
                        )
                        if m > 128:
                            nc.tensor.matmul(
                                pb[:m - 128, :nt],
                                lhsT=wsb[:, offs[p] + 128: offs[p] + m],
                                rhs=rhs,
                                start=(p == b_ps[0]),
                                stop=(p == b_ps[-1]),
                            )
                    ma1 = min(Ms[a_ps[0]], 128)
                    eva = evpool.tile([128, 512], f32, tag="eva")
                    nc.vector.tensor_copy(eva[:ma1, :nt], pa[:ma1, :nt])
                    nc.sync.dma_start(
                        out=out_d[b, 0:ma1, t0:t0 + nt], in_=eva[:ma1, :nt]
                    )
                    if mb_max:
                        evb = evpool.tile([128, 512], f32, tag="evb")
                        nc.vector.tensor_copy(evb[:mb_max, :nt], pb[:mb_max, :nt])
                        nc.sync.dma_start(
                            out=out_d[b, 128:128 + mb_max, t0:t0 + nt],
                            in_=evb[:mb_max, :nt],
                        )
                    t0 += nt
    nc.compile()
    return nc


def _ensure_trace_shims():
    """If run_bass_kernel_spmd is invoked with tracing enabled (e.g. via
    BASS_TRACE=1) it imports antenv.axon_hooks and uploads artifacts to a
    bucket; neither exists in a bare container.  Register a working NTFF
    hook (ctypes into the axon .so) and a no-op uploader so the trace path
    degrades gracefully instead of crashing."""
    import sys

    try:
        import antenv.axon_hooks  # noqa: F401
    except ImportError:
        import contextlib
        import ctypes
        import types

        hook = None
        try:
            lib = ctypes.CDLL("/opt/axon/libaxon_pjrt.so")
            if hasattr(lib, "axon_start_nrt_profile"):
                lib.axon_start_nrt_profile.argtypes = [
                    ctypes.POINTER(ctypes.c_int64),
                    ctypes.c_size_t,
                ]
                lib.axon_start_nrt_profile.restype = ctypes.c_int64
                lib.axon_stop_nrt_profile.argtypes = [ctypes.c_char_p]
                lib.axon_stop_nrt_profile.restype = ctypes.c_int64

                @contextlib.contextmanager
                def _hook(output_dir, device_ids):
                    import jax

                    jax.devices()
                    if device_ids:
                        ids = (ctypes.c_int64 * len(device_ids))(*device_ids)
                        rc = lib.axon_start_nrt_profile(ids, len(device_ids))
                    else:
                        rc = lib.axon_start_nrt_profile(None, 0)
                    if rc != 0:
                        raise RuntimeError(f"axon_start_nrt_profile rc={rc}")
                    try:
                        yield
                    finally:
                        lib.axon_stop_nrt_profile(str(output_dir).encode())

                hook = _hook
        except OSError:
            pass
        mod = types.ModuleType("antenv.axon_hooks")
        mod.get_axon_ntff_profile_hook = lambda: hook
        mod.set_axon_ntff_profile_hook = lambda h: None
        sys.modules["antenv.axon_hooks"] = mod

    try:
        import concourse.bass_utils as _bu

        _orig_upload = _bu.upload_artifacts

        def _safe_upload(tmpdir):
            try:
                return _orig_upload(tmpdir)
            except Exception:
                return "local://unavailable"

        if not getattr(_bu, "_safe_upload_installed", False):
            _bu.upload_artifacts = _safe_upload
            _bu._safe_upload_installed = True
    except Exception:
        pass


def kernel(x, kernels):
    _ensure_trace_shims()
    from concourse.bass_utils import run_bass_kernel_spmd

    xt, wt, keep, Ms, offs, C, U, T_out, nbins = _host_prep(x, kernels)
    B = xt.shape[0]
    assert B % N_CORES == 0
    b_per = B // N_CORES

    key = (b_per, C, U, T_out, tuple(keep), tuple(Ms))
    if key not in _prog_cache:
        _prog_cache[key] = _build_program(b_per, C, U, T_out, keep, Ms, offs)
    nc = _prog_cache[key]

    in_maps = [
        {"xt": xt[c * b_per:(c + 1) * b_per], "wt": wt} for c in range(N_CORES)
    ]
    res = run_bass_kernel_spmd(nc, in_maps, list(range(N_CORES)))
    parts = [res.results[c]["out"] for c in range(N_CORES)]
    out = np.concatenate(parts, axis=0)  # (B, C, T_out)
    return np.ascontiguousarray(
        out.reshape(B, nbins, 2, T_out).transpose(0, 2, 1, 3)
    )

